# revision 1
# baseline (speedup 1.0000x reference)
"""Trainium2 Bass kernel for nn_LocallyDense: gather -> 16 group-GEMMs -> Conv1D(k=32) -> LeakyReLU.

Strategy: data-parallel over batch (32 -> 4 per core on 8 cores).
Host staging: apply the idx permutation + transpose while sharding (pure data
movement), so the device sees dense GEMMs only:
  stage 1: h[b] = x_perm[b] @ W[g] per group, computed as hT[d, (b,pos)]
  stage 2: y[b] = LeakyReLU(conv1d(h[b])) as a K=8192 GEMM accumulated in PSUM.
All matmuls in float32r (full-rate at moving-dim >= 256, ~1.5e-4 rel err).
Device output layout y[b, o, t]; host transposes back to [b, t, o].
"""
import numpy as np

import concourse.bass as bass
import concourse.mybir as mybir
import concourse.tile as tile
from concourse.alu_op_type import AluOpType
from concourse import bacc
from concourse.bass_utils import run_bass_kernel_spmd

B, N, F, G, S, D = 32, 1024, 512, 16, 64, 256
KC, O = 32, 512            # conv kernel taps, conv out channels
T = N - KC + 1             # 993 valid conv outputs
NCORES = 8
BPC = B // NCORES          # batches per core
NEG_SLOPE = 0.2
F32 = mybir.dt.float32
F32R = mybir.dt.float32r

TRACE = False              # test.py flips this to get a profile
STAGES = (1, 2)            # bench knob: which stages to emit
_cache = {}


def _build():
    nc = bacc.Bacc("TRN2", target_bir_lowering=False, debug=False,
                   num_devices=NCORES)
    xpt_d = nc.dram_tensor("xpt", [BPC, F, N], F32, kind="ExternalInput").ap()
    w_d = nc.dram_tensor("w", [G, F, D], F32, kind="ExternalInput").ap()
    b_d = nc.dram_tensor("b", [G, D], F32, kind="ExternalInput").ap()
    cw_d = nc.dram_tensor("cw", [4, KC * 2, 128, 128], F32,
                          kind="ExternalInput").ap()
    cb_d = nc.dram_tensor("cb", [O], F32, kind="ExternalOutput"
                          if False else "ExternalInput").ap()
    y_d = nc.dram_tensor("y", [BPC, O, T], F32, kind="ExternalOutput").ap()

    FKT = F // 128           # 4 k-tiles over F
    KK = KC * 2              # 64 k-chunks over (tap, d-half)
    with tile.TileContext(nc) as tc:
        with tc.tile_pool(name="xpt", bufs=4) as p_xpt, \
             tc.tile_pool(name="wg", bufs=6) as p_w, \
             tc.tile_pool(name="ht", bufs=1) as p_ht, \
             tc.tile_pool(name="bias", bufs=1) as p_bias, \
             tc.tile_pool(name="cw", bufs=2) as p_cw, \
             tc.tile_pool(name="yout", bufs=8) as p_out:

            # biases: b[g, m*128+p] -> b_sb[p, g*2+m]; conv_b[m*128+p] -> cb_sb[p, m]
            b_sb = p_bias.tile([128, G * 2], F32)
            nc.sync.dma_start(b_sb[:], b_d.rearrange("g (m p) -> p (g m)", p=128))
            cb_sb = p_bias.tile([128, 4], F32)
            nc.sync.dma_start(cb_sb[:], cb_d.rearrange("(m p) -> p m", p=128))

            # x permuted+transposed: per f-ktile a [128, BPC*N] tile, b-major cols
            xpt_sb = []
            for kt in range(FKT):
                t = p_xpt.tile([128, BPC * N], F32R, tag="xpt")
                for bb in range(BPC):
                    nc.sync.dma_start(
                        t[:, bb * N:(bb + 1) * N],
                        xpt_d[bb, kt * 128:(kt + 1) * 128, :].bitcast(F32R))
                xpt_sb.append(t)

            # conv weights per o-tile m: [128, KK*128], chunk k holds lhsT
            cw_sb = []
            for m in range(4):
                t = p_cw.tile([128, KK * 128], F32R, tag="cw")
                nc.sync.dma_start(
                    t[:].rearrange("p (k o) -> p k o", k=KK),
                    cw_d[m].bitcast(F32R).rearrange("k p o -> p k o"))
                cw_sb.append(t)

            # hT[m]: [128, BPC*N] float32r, d-half m on partitions
            ht_sb = [p_ht.tile([128, BPC * N], F32R, tag=f"ht{m}", name=f"ht{m}")
                     for m in range(2)]

            # ---------------- stage 1: group GEMMs ----------------
            if 1 in STAGES:
              with tc.tile_pool(name="ps1", bufs=6, space="PSUM") as p_ps1:
                  for g in range(G):
                      w_sb = p_w.tile([128, FKT * D], F32R, tag="wg")
                      nc.sync.dma_start(
                          w_sb[:].rearrange("p (kt d) -> p kt d", kt=FKT),
                          w_d[g].bitcast(F32R).rearrange("(kt p) d -> p kt d", p=128))
                      for m in range(2):
                          ps = p_ps1.tile([128, BPC * S], F32, tag="ps1")
                          for kt in range(FKT):
                              rhs = xpt_sb[kt][:].rearrange(
                                  "p (b t) -> p b t", b=BPC)[:, :, g * S:(g + 1) * S]
                              nc.tensor.matmul(
                                  ps[:], w_sb[:, kt * D + m * 128: kt * D + (m + 1) * 128],
                                  rhs, start=(kt == 0), stop=(kt == FKT - 1))
                          dest = ht_sb[m][:].rearrange(
                              "p (b t) -> p b t", b=BPC)[:, :, g * S:(g + 1) * S]
                          # h = psum + bias[g, m-half] (copy + rounds to f32r)
                          nc.vector.tensor_scalar_add(
                              dest, ps[:], b_sb[:, g * 2 + m: g * 2 + m + 1])

            # ---------------- stage 2: conv as GEMM ----------------
            ntiles = []
            for bb in range(BPC):
                ntiles.append((bb, 0, 512))
                # fp32r ISA requires even element counts; overlap col 511
                ntiles.append((bb, 511, 482))
            if 2 not in STAGES:
                ntiles = []
            with tc.tile_pool(name="ps2", bufs=8, space="PSUM") as p_ps2:
                for m in range(4 if 2 in STAGES else 0):
                    pss = [p_ps2.tile([128, 512], F32, tag="ps2", name=f"ps2_{m}_{j}")
                           for j in range(len(ntiles))]
                    for k in range(KK):
                        tap, dh = k // 2, k % 2
                        lhsT = cw_sb[m][:, k * 128:(k + 1) * 128]
                        for j, (bb, t0, nn) in enumerate(ntiles):
                            rhs = ht_sb[dh][:, bb * N + tap + t0:
                                            bb * N + tap + t0 + nn]
                            nc.tensor.matmul(pss[j][:, :nn], lhsT, rhs,
                                             start=(k == 0), stop=(k == KK - 1))
                    for j, (bb, t0, nn) in enumerate(ntiles):
                        y_sb = p_out.tile([128, 512], F32, tag="yout")
                        nc.scalar.activation(
                            y_sb[:, :nn], pss[j][:, :nn],
                            mybir.ActivationFunctionType.Identity,
                            bias=cb_sb[:, m:m + 1])
                        nc.vector.scalar_tensor_tensor(
                            y_sb[:, :nn], y_sb[:, :nn], NEG_SLOPE, y_sb[:, :nn],
                            AluOpType.mult, AluOpType.max)
                        nc.sync.dma_start(
                            y_d[bb, m * 128:(m + 1) * 128, t0:t0 + nn],
                            y_sb[:, :nn])
    nc.compile()
    return nc


def kernel(x, idx, W, b, conv_w, conv_b):
    x = np.asarray(x); idx = np.asarray(idx); W = np.asarray(W)
    b = np.asarray(b); conv_w = np.asarray(conv_w); conv_b = np.asarray(conv_b)
    if "nc" not in _cache:
        _cache["nc"] = _build()
    nc = _cache["nc"]

    idx_flat = idx.reshape(-1).astype(np.int64)
    # permute + transpose: xpt[b, f, p] = x[b, idx_flat[p], f]
    xpt = np.ascontiguousarray(x[:, idx_flat, :].transpose(0, 2, 1))
    # conv_w[tap, d, o] -> cw[m, (tap, dh), p, o]
    cw = np.ascontiguousarray(
        conv_w.reshape(KC, 2, 128, 4, 128).transpose(3, 0, 1, 2, 4)
    ).reshape(4, KC * 2, 128, 128)
    W_c = np.ascontiguousarray(W)
    b_c = np.ascontiguousarray(b)
    cb_c = np.ascontiguousarray(conv_b)

    in_maps = []
    for c in range(NCORES):
        in_maps.append({
            "xpt": xpt[c * BPC:(c + 1) * BPC],
            "w": W_c, "b": b_c, "cw": cw, "cb": cb_c,
        })
    res = run_bass_kernel_spmd(nc, in_maps, core_ids=list(range(NCORES)),
                               trace=TRACE)
    if TRACE and res.exec_time_ns is not None:
        print(f"HW exec time: {res.exec_time_ns} ns")
        if res.instructions_and_trace is not None:
            print("trace:", res.instructions_and_trace[1])
    y = np.concatenate([r["y"] for r in res.results], axis=0)  # [B, O, T]
    return np.ascontiguousarray(y.transpose(0, 2, 1)).astype(np.float32)



# revision 3
# speedup vs baseline: 1.1149x; 1.1149x over previous
"""Trainium2 Bass kernel for nn_LocallyDense: gather -> 16 group-GEMMs -> Conv1D(k=32) -> LeakyReLU.

Data-parallel over batch (32 -> 4 per core on 8 cores). Host staging applies
the idx permutation + transpose so the device sees dense GEMMs only. All
matmuls in bf16 (error ~2e-3 << 2e-2 tolerance), which halves DMA traffic
versus f32 and has no moving-dim-size constraints.

Schedule (DMA transfers are serialized in the timeline model, so order
matters): biases, xpt[bb=0], all 16 group weights, conv weights m=0 (split
into 8 chunks so stage 2 can start while they stream), xpt[bb=1..3], conv
weights m=1..3. Stage 1 for bb=0 runs while its weights stream; stage 2 for
bb=0 starts right after, overlapping the remaining input DMAs; stage 1 for
bb=1..3 is interleaved between stage-2 blocks.
"""
import numpy as np
import ml_dtypes

import concourse.bass as bass
import concourse.mybir as mybir
import concourse.tile as tile
from concourse.alu_op_type import AluOpType
from concourse import bacc
from concourse.bass_utils import run_bass_kernel_spmd

B, N, F, G, S, D = 32, 1024, 512, 16, 64, 256
KC, O = 32, 512            # conv kernel taps, conv out channels
T = N - KC + 1             # 993 valid conv outputs
NCORES = 8
BPC = B // NCORES          # batches per core
NEG_SLOPE = 0.2
F32 = mybir.dt.float32
BF16 = mybir.dt.bfloat16
BF = ml_dtypes.bfloat16

FKT = F // 128             # 4 k-tiles over F
KK = KC * 2                # 64 k-chunks over (tap, d-half)
NTILES = [(0, 512), (512, T - 512)]   # stage-2 output column tiles per batch

TRACE = False
_cache = {}


def _build():
    nc = bacc.Bacc("TRN2", target_bir_lowering=False, debug=False,
                   num_devices=NCORES)
    # host layouts chosen for contiguous-chunk DMA (>=512B per run)
    xpt_d = nc.dram_tensor("xpt", [BPC, FKT, 128, N], BF16,
                           kind="ExternalInput").ap()
    w_d = nc.dram_tensor("w", [G, 128, FKT, D], BF16,
                         kind="ExternalInput").ap()
    b_d = nc.dram_tensor("b", [G, D], F32, kind="ExternalInput").ap()
    cw_d = nc.dram_tensor("cw", [4, 128, KK, 128], BF16,
                          kind="ExternalInput").ap()
    cb_d = nc.dram_tensor("cb", [O], F32, kind="ExternalInput").ap()
    y_d = nc.dram_tensor("y", [BPC, O, T], BF16, kind="ExternalOutput").ap()

    with tile.TileContext(nc) as tc:
        with tc.tile_pool(name="xpt", bufs=BPC * FKT) as p_xpt, \
             tc.tile_pool(name="wg", bufs=G) as p_w, \
             tc.tile_pool(name="ht", bufs=BPC * 2) as p_ht, \
             tc.tile_pool(name="bias", bufs=2) as p_bias, \
             tc.tile_pool(name="cw", bufs=4) as p_cw, \
             tc.tile_pool(name="yout", bufs=6) as p_out:

            # ---- input DMAs, hoisted in execution-priority order ----
            # biases: b[g, m*128+p] -> b_sb[p, g*2+m]; conv_b -> cb_sb[p, m]
            b_sb = p_bias.tile([128, G * 2], F32)
            nc.sync.dma_start(b_sb[:], b_d.rearrange("g (m p) -> p (g m)", p=128))
            cb_sb = p_bias.tile([128, 4], F32)
            nc.sync.dma_start(cb_sb[:], cb_d.rearrange("(m p) -> p m", p=128))

            xpt_sb = [[p_xpt.tile([128, N], BF16, tag="xpt", name=f"xpt{bb}_{kt}")
                       for kt in range(FKT)] for bb in range(BPC)]
            for kt in range(FKT):
                nc.sync.dma_start(xpt_sb[0][kt][:], xpt_d[0, kt])

            w_sb = [p_w.tile([128, FKT, D], BF16, tag="wg", name=f"w{g}")
                    for g in range(G)]
            for g in range(G):
                nc.sync.dma_start(w_sb[g][:], w_d[g])

            cw_sb = [p_cw.tile([128, KK, 128], BF16, tag="cw", name=f"cw{m}")
                     for m in range(4)]
            KG = 8  # chunks per cw sub-DMA
            for kg in range(KK // KG):
                nc.sync.dma_start(cw_sb[0][:, kg * KG:(kg + 1) * KG, :],
                                  cw_d[0][:, kg * KG:(kg + 1) * KG, :])
            for bb in range(1, BPC):
                for kt in range(FKT):
                    nc.sync.dma_start(xpt_sb[bb][kt][:], xpt_d[bb, kt])
            for m in range(1, 4):
                for kg in range(KK // KG):
                    nc.sync.dma_start(cw_sb[m][:, kg * KG:(kg + 1) * KG, :],
                                      cw_d[m][:, kg * KG:(kg + 1) * KG, :])

            # ht[bb][mh]: stage-1 output, d-half mh on partitions
            ht_sb = [[p_ht.tile([128, N], BF16, tag="ht", name=f"ht{bb}_{mh}")
                      for mh in range(2)] for bb in range(BPC)]

            def stage1(bb, p_ps1):
                for g in range(G):
                    for mh in range(2):
                        ps = p_ps1.tile([128, S], F32, tag="ps1")
                        for kt in range(FKT):
                            nc.tensor.matmul(
                                ps[:], w_sb[g][:, kt, mh * 128:(mh + 1) * 128],
                                xpt_sb[bb][kt][:, g * S:(g + 1) * S],
                                start=(kt == 0), stop=(kt == FKT - 1))
                        nc.vector.tensor_scalar_add(
                            ht_sb[bb][mh][:, g * S:(g + 1) * S], ps[:],
                            b_sb[:, g * 2 + mh: g * 2 + mh + 1])

            def stage2_block(bb, m, p_ps2):
                for t0, nn in NTILES:
                    ps = p_ps2.tile([128, 512], F32, tag="ps2")
                    for k in range(KK):
                        tap, dh = k // 2, k % 2
                        nc.tensor.matmul(
                            ps[:, :nn], cw_sb[m][:, k, :],
                            ht_sb[bb][dh][:, tap + t0: tap + t0 + nn],
                            start=(k == 0), stop=(k == KK - 1))
                    y_sb = p_out.tile([128, 512], BF16, tag="yout")
                    nc.scalar.activation(
                        y_sb[:, :nn], ps[:, :nn],
                        mybir.ActivationFunctionType.Identity,
                        bias=cb_sb[:, m:m + 1])
                    nc.vector.scalar_tensor_tensor(
                        y_sb[:, :nn], y_sb[:, :nn], NEG_SLOPE, y_sb[:, :nn],
                        AluOpType.mult, AluOpType.max)
                    nc.sync.dma_start(
                        y_d[bb, m * 128:(m + 1) * 128, t0:t0 + nn],
                        y_sb[:, :nn])

            with tc.tile_pool(name="ps1", bufs=4, space="PSUM") as p_ps1, \
                 tc.tile_pool(name="ps2", bufs=4, space="PSUM") as p_ps2:
                stage1(0, p_ps1)
                # interleave stage-1 of bb=1..3 between early stage-2 blocks
                stage2_block(0, 0, p_ps2)
                stage1(1, p_ps1)
                stage2_block(0, 1, p_ps2)
                stage1(2, p_ps1)
                stage2_block(0, 2, p_ps2)
                stage1(3, p_ps1)
                stage2_block(0, 3, p_ps2)
                for bb in range(1, BPC):
                    for m in range(4):
                        stage2_block(bb, m, p_ps2)
    nc.compile()
    return nc


def kernel(x, idx, W, b, conv_w, conv_b):
    x = np.asarray(x); idx = np.asarray(idx); W = np.asarray(W)
    b = np.asarray(b); conv_w = np.asarray(conv_w); conv_b = np.asarray(conv_b)
    if "nc" not in _cache:
        _cache["nc"] = _build()
    nc = _cache["nc"]

    idx_flat = idx.reshape(-1).astype(np.int64)
    # xpt[b, kt, p, n] = x[b, idx_flat[n], kt*128+p]
    xpt = np.ascontiguousarray(
        x[:, idx_flat, :].transpose(0, 2, 1).reshape(B, FKT, 128, N)
    ).astype(BF)
    # w[g, p, kt, d] = W[g, kt*128+p, d]
    wg = np.ascontiguousarray(
        W.reshape(G, FKT, 128, D).transpose(0, 2, 1, 3)).astype(BF)
    # cw[m, p, k=(tap,dh), o] = conv_w[tap, dh*128+p, m*128+o]
    cw = np.ascontiguousarray(
        conv_w.reshape(KC, 2, 128, 4, 128).transpose(3, 2, 0, 1, 4)
        .reshape(4, 128, KK, 128)).astype(BF)
    b_c = np.ascontiguousarray(b).astype(np.float32)
    cb_c = np.ascontiguousarray(conv_b).astype(np.float32)

    in_maps = []
    for c in range(NCORES):
        in_maps.append({
            "xpt": xpt[c * BPC:(c + 1) * BPC],
            "w": wg, "b": b_c, "cw": cw, "cb": cb_c,
        })
    res = run_bass_kernel_spmd(nc, in_maps, core_ids=list(range(NCORES)),
                               trace=TRACE)
    if TRACE and res.exec_time_ns is not None:
        print(f"HW exec time: {res.exec_time_ns} ns")
    y = np.concatenate([r["y"] for r in res.results], axis=0)  # [B, O, T] bf16
    return np.ascontiguousarray(
        y.transpose(0, 2, 1).astype(np.float32))


# revision 7
# speedup vs baseline: 1.7174x; 1.5404x over previous
"""Trainium2 Bass kernel for nn_LocallyDense: gather -> 16 group-GEMMs -> Conv1D(k=32) -> LeakyReLU.

Data-parallel over batch (32 -> 4 per core). Host applies the idx permutation;
stage 1 computes h = x_perm @ W_g per group in bf16.

The k=32 conv is computed with a 2-level Karatsuba decomposition of the tap
dimension: instead of 32 tap-GEMMs per output, 9 sub-correlations of 8 taps
over stride-4 subsampled/differenced sequences (datasets), cutting tensor-
engine work to 9/16 of direct. Derivation (per d-channel, position t):
  level 1: E_s=h[2s+1], P_s=h[2s]-h[2s+1], Q_s=h[2s+2]-h[2s+1]
           A=corr(E,a), B=corr(P,e), C=corr(Q,o) with a=w0+w1 pairs, e/o =
           even/odd taps; y_2u=A_u+B_u, y_2u+1=A_u+C_u
  level 2: the same split applied to each of A,B,C -> 9 corrs of 8 taps.
The 6 needed sums R[i][A2]+R[i][B2], R[i][A2]+R[i][C2] are built in PSUM by
snapshotting the A2 partial (ACT copy psA->psB) and accumulating B2/C2 on
top, so no extra matmul work. Final y phases are DVE adds of PSUM pairs with
the conv bias fused, then LeakyReLU, all in bf16 (rel err ~6e-3 << 2e-2).

DMA transfers are serialized in the timeline model: inputs stream on the SP
queue in execution-priority order; y outputs go out on the Activation queue
to avoid head-of-line blocking. Conv combo weights (2.25x the direct conv
weights) are streamed per (batch-pair, m, corr) and re-fetched for the second
batch pair to bound SBUF.
"""
import numpy as np
import ml_dtypes

import concourse.bass as bass
import concourse.mybir as mybir
import concourse.tile as tile
from concourse.alu_op_type import AluOpType
from concourse import bacc
from concourse.bass_utils import run_bass_kernel_spmd

B, N, F, G, S, D = 32, 1024, 512, 16, 64, 256
KC, O = 32, 512
T = N - KC + 1             # 993
NCORES = 8
BPC = B // NCORES          # 4
NEG_SLOPE = 0.2
F32 = mybir.dt.float32
BF16 = mybir.dt.bfloat16
BF = ml_dtypes.bfloat16

FKT = F // 128             # 4
NP = 1032                  # padded h length
U2 = 249                   # level-2 correlation outputs (4*249 >= 993+3)
NCORR = 9
K2 = 16                    # chunks per corr: 8 taps x 2 d-halves

TRACE = False
_cache = {}


def _sv(t, start, step, n):
    """Strided column view [128, n]: cols start, start+step, ..."""
    return t[:, start:start + step * n].rearrange(
        "p (r f) -> p r f", f=step)[:, :, 0]


def _build():
    nc = bacc.Bacc("TRN2", target_bir_lowering=False, debug=False,
                   num_devices=NCORES)
    xpt_d = nc.dram_tensor("xpt", [BPC, FKT, 128, N], BF16,
                           kind="ExternalInput").ap()
    w_d = nc.dram_tensor("w", [G, 128, FKT, D], BF16,
                         kind="ExternalInput").ap()
    b_d = nc.dram_tensor("b", [G, D], F32, kind="ExternalInput").ap()
    cw_d = nc.dram_tensor("cw", [4, 128, NCORR, K2, 128], BF16,
                          kind="ExternalInput").ap()
    cb_d = nc.dram_tensor("cb", [O], F32, kind="ExternalInput").ap()
    y_d = nc.dram_tensor("y", [BPC, O, T], BF16, kind="ExternalOutput").ap()

    with tile.TileContext(nc) as tc:
        with tc.tile_pool(name="xpt", bufs=2 * FKT) as p_xpt, \
             tc.tile_pool(name="wg", bufs=G) as p_w, \
             tc.tile_pool(name="ht", bufs=BPC * 2) as p_ht, \
             tc.tile_pool(name="pq", bufs=BPC * 4) as p_pq, \
             tc.tile_pool(name="df", bufs=BPC * 12) as p_df, \
             tc.tile_pool(name="bias", bufs=2) as p_bias, \
             tc.tile_pool(name="cw", bufs=12) as p_cw, \
             tc.tile_pool(name="ss", bufs=16) as p_ss, \
             tc.tile_pool(name="ys", bufs=3) as p_ys, \
             tc.tile_pool(name="yo", bufs=3) as p_yo:

            # ---------------- input DMAs (SP queue, priority order) --------
            xpt_sb = {}
            for bb in range(2):
                for kt in range(FKT):
                    t_ = p_xpt.tile([128, N], BF16, tag="xpt",
                                    name=f"xpt{bb}_{kt}")
                    nc.sync.dma_start(t_[:], xpt_d[bb, kt])
                    xpt_sb[(bb, kt)] = t_
            w_sb = [p_w.tile([128, FKT, D], BF16, tag="wg", name=f"w{g}")
                    for g in range(G)]
            for g in range(G):
                nc.sync.dma_start(w_sb[g][:], w_d[g])
            b_sb = p_bias.tile([128, G * 2], F32)
            nc.sync.dma_start(b_sb[:], b_d.rearrange("g (m p) -> p (g m)", p=128))
            cb_sb = p_bias.tile([128, 4], F32)
            nc.sync.dma_start(cb_sb[:], cb_d.rearrange("(m p) -> p m", p=128))

            def fetch_cw(pair, m):
                out = []
                for c in range(NCORR):
                    t_ = p_cw.tile([128, K2, 128], BF16, tag="cw",
                                   name=f"cw{pair}_{m}_{c}")
                    nc.sync.dma_start(t_[:], cw_d[m][:, c])
                    out.append(t_)
                return out

            cw_sb = {}
            cw_sb[(0, 0)] = fetch_cw(0, 0)

            # xpt for bb2/3 after the first conv weights
            for bb in range(2, BPC):
                for kt in range(FKT):
                    t_ = p_xpt.tile([128, N], BF16, tag="xpt",
                                    name=f"xpt{bb}_{kt}")
                    nc.sync.dma_start(t_[:], xpt_d[bb, kt])
                    xpt_sb[(bb, kt)] = t_
            for m in range(1, 4):
                cw_sb[(0, m)] = fetch_cw(0, m)
            for m in range(4):
                cw_sb[(1, m)] = fetch_cw(1, m)

            # ---------------- per-batch state ------------------------------
            ht = {}     # (bb, dh) -> [128, NP] bf16 (padded h)
            pq = {}     # (bb, dh, 0/1) -> P/Q [128, 514]
            dfs = {}    # (bb, dh, i, j) -> diff tiles [128, 256], j in 1,2

            def stage1(bb, p_ps1):
                for dh in range(2):
                    t_ = p_ht.tile([128, NP], BF16, tag="ht",
                                   name=f"ht{bb}_{dh}")
                    ht[(bb, dh)] = t_
                    nc.vector.memset(t_[:, N:NP], 0.0)
                for g in range(G):
                    for dh in range(2):
                        ps_full = p_ps1.tile([128, 256], F32, tag="ps2",
                                             name=f"ps1_{bb}_{g}_{dh}")
                        ps = ps_full[:, :S]
                        for kt in range(FKT):
                            nc.tensor.matmul(
                                ps, w_sb[g][:, kt, dh * 128:(dh + 1) * 128],
                                xpt_sb[(bb, kt)][:, g * S:(g + 1) * S],
                                start=(kt == 0), stop=(kt == FKT - 1))
                        nc.vector.tensor_scalar_add(
                            ht[(bb, dh)][:, g * S:(g + 1) * S], ps,
                            b_sb[:, g * 2 + dh: g * 2 + dh + 1])

            def sub(out, a, b_):
                nc.vector.scalar_tensor_tensor(
                    out, a, 1.0, b_, AluOpType.mult, AluOpType.subtract)

            def transforms(bb):
                for dh in range(2):
                    h_ = ht[(bb, dh)]
                    P = p_pq.tile([128, 514], BF16, tag="pq",
                                  name=f"P{bb}_{dh}")
                    Q = p_pq.tile([128, 514], BF16, tag="pq",
                                  name=f"Q{bb}_{dh}")
                    sub(P[:], _sv(h_, 0, 2, 514), _sv(h_, 1, 2, 514))
                    sub(Q[:], _sv(h_, 2, 2, 514), _sv(h_, 1, 2, 514))
                    pq[(bb, dh, 0)] = P
                    pq[(bb, dh, 1)] = Q
                    # level-2 diff datasets per lineage: (i, j=1 (B2), j=2 (C2))
                    for i, src, st0, stp in ((0, h_, 1, 4), (1, P, 0, 2),
                                             (2, Q, 0, 2)):
                        for j, off in ((1, 0), (2, 2 * (stp // 2))):
                            dt_ = p_df.tile([128, 256], BF16, tag="df",
                                            name=f"d{bb}_{dh}_{i}_{j}")
                            if i == 0:
                                va = _sv(src, 1 if j == 1 else 5, 4, 256)
                                vb = _sv(src, 3, 4, 256)
                            else:
                                va = _sv(src, 0 if j == 1 else 2, 2, 256)
                                vb = _sv(src, 1, 2, 256)
                            sub(dt_[:], va, vb)
                            dfs[(bb, dh, i, j)] = dt_

            def rhs(bb, dh, i, j, v2):
                if j == 0:
                    if i == 0:
                        return _sv(ht[(bb, dh)], 3 + 4 * v2, 4, U2)
                    return _sv(pq[(bb, dh, i - 1)], 1 + 2 * v2, 2, U2)
                return dfs[(bb, dh, i, j)][:, v2:v2 + U2]

            def s2_block(pair, m, bb, p_ps2):
                """9 GEMM-sets for one (batch, m). psA_i accumulates A2 then
                C2 (odd sums); A2 partials are snapshotted to SBUF (sA2_i)
                before C2 lands; B2 runs as its own clean PSUM group. Each
                PSUM bank is written only by the PE, so there are no
                cross-engine PSUM write races. Phases are assembled on DVE
                with at most one PSUM operand per op, conv bias fused."""
                cwt = cw_sb[(pair, m)]
                psA, psB, sA2 = [], [], []
                for i in range(3):
                    ps = p_ps2.tile([128, 256], F32, tag="ps2",
                                    name=f"psA{pair}_{m}_{bb}_{i}")
                    psA.append(ps)
                    c = 3 * i + 0
                    for k2 in range(K2):
                        v2, dh = k2 // 2, k2 % 2
                        nc.tensor.matmul(
                            ps[:, :U2], cwt[c][:, k2, :], rhs(bb, dh, i, 0, v2),
                            start=(k2 == 0), stop=False, skip_group_check=True)
                    s_ = p_ss.tile([128, 256], F32, tag="ss",
                                   name=f"sA2_{pair}_{m}_{bb}_{i}")
                    nc.scalar.activation(s_[:, :U2], ps[:, :U2],
                                         mybir.ActivationFunctionType.Copy)
                    sA2.append(s_)
                for i in range(3):
                    pb = p_ps2.tile([128, 256], F32, tag="ps2",
                                    name=f"psB{pair}_{m}_{bb}_{i}")
                    psB.append(pb)
                    c = 3 * i + 1
                    for k2 in range(K2):
                        v2, dh = k2 // 2, k2 % 2
                        nc.tensor.matmul(
                            pb[:, :U2], cwt[c][:, k2, :], rhs(bb, dh, i, 1, v2),
                            start=(k2 == 0), stop=(k2 == K2 - 1))
                for i in range(3):
                    c = 3 * i + 2
                    for k2 in range(K2):
                        v2, dh = k2 // 2, k2 % 2
                        nc.tensor.matmul(
                            psA[i][:, :U2], cwt[c][:, k2, :],
                            rhs(bb, dh, i, 2, v2),
                            start=False, stop=(k2 == K2 - 1),
                            skip_group_check=True)
                # sAC = psA[0] (used twice; DVE allows only one PSUM operand)
                sAC = p_ss.tile([128, 256], F32, tag="ss",
                                name=f"sAC{pair}_{m}_{bb}")
                nc.scalar.activation(sAC[:, :U2], psA[0][:, :U2],
                                     mybir.ActivationFunctionType.Copy)
                # phases: y4u+0 = (sA2_0+sA2_1) + B2_0 + B2_1
                #         y4u+1 = (sA2_0+sA2_2) + B2_0 + B2_2
                #         y4u+2 = (A2_0+C2_0) + (A2_1+C2_1) = sAC + psA[1]
                #         y4u+3 = sAC + psA[2]
                y_sb = p_ys.tile([128, 1000], F32, tag="ys")
                cbm = cb_sb[:, m:m + 1]

                def stt(out, a, scal, b_):
                    nc.vector.scalar_tensor_tensor(
                        out, a, scal, b_, AluOpType.add, AluOpType.add)

                tAB = p_ss.tile([128, 256], F32, tag="ss",
                                name=f"tAB{pair}_{m}_{bb}")
                tAC = p_ss.tile([128, 256], F32, tag="ss",
                                name=f"tAC{pair}_{m}_{bb}")
                stt(tAB[:, :U2], sA2[0][:, :U2], 0.0, sA2[1][:, :U2])
                stt(tAC[:, :U2], sA2[0][:, :U2], 0.0, sA2[2][:, :U2])
                v0 = p_ss.tile([128, 256], F32, tag="ss",
                               name=f"v0_{pair}_{m}_{bb}")
                v1 = p_ss.tile([128, 256], F32, tag="ss",
                               name=f"v1_{pair}_{m}_{bb}")
                stt(v0[:, :U2], tAB[:, :U2], cbm, psB[0][:, :U2])
                stt(_sv(y_sb, 0, 4, U2), v0[:, :U2], 0.0, psB[1][:, :U2])
                stt(v1[:, :U2], tAC[:, :U2], cbm, psB[0][:, :U2])
                stt(_sv(y_sb, 1, 4, U2), v1[:, :U2], 0.0, psB[2][:, :U2])
                stt(_sv(y_sb, 2, 4, U2), sAC[:, :U2], cbm, psA[1][:, :U2])
                stt(_sv(y_sb, 3, 4, U2), sAC[:, :U2], cbm, psA[2][:, :U2])
                y_out = p_yo.tile([128, 1000], BF16, tag="yo")
                nc.vector.scalar_tensor_tensor(
                    y_out[:, :T], y_sb[:, :T], NEG_SLOPE, y_sb[:, :T],
                    AluOpType.mult, AluOpType.max)
                nc.scalar.dma_start(y_d[bb, m * 128:(m + 1) * 128, :],
                                    y_out[:, :T])

            with tc.tile_pool(name="ps2", bufs=8, space="PSUM") as p_ps2:
                stage1(0, p_ps2)
                transforms(0)
                stage1(1, p_ps2)
                transforms(1)
                s2_block(0, 0, 0, p_ps2)
                s2_block(0, 0, 1, p_ps2)
                stage1(2, p_ps2)
                transforms(2)
                s2_block(0, 1, 0, p_ps2)
                s2_block(0, 1, 1, p_ps2)
                stage1(3, p_ps2)
                transforms(3)
                for m in range(2, 4):
                    s2_block(0, m, 0, p_ps2)
                    s2_block(0, m, 1, p_ps2)
                for m in range(4):
                    s2_block(1, m, 2, p_ps2)
                    s2_block(1, m, 3, p_ps2)
    nc.compile()
    return nc


def kernel(x, idx, W, b, conv_w, conv_b):
    x = np.asarray(x); idx = np.asarray(idx); W = np.asarray(W)
    b = np.asarray(b); conv_w = np.asarray(conv_w); conv_b = np.asarray(conv_b)
    if "nc" not in _cache:
        _cache["nc"] = _build()
    nc = _cache["nc"]

    idx_flat = idx.reshape(-1).astype(np.int64)
    xpt = np.ascontiguousarray(
        x[:, idx_flat, :].transpose(0, 2, 1).reshape(B, FKT, 128, N)
    ).astype(BF)
    wg = np.ascontiguousarray(
        W.reshape(G, FKT, 128, D).transpose(0, 2, 1, 3)).astype(BF)

    # 9 combo-weight sets, 8 taps each (Karatsuba level-2 tap combos)
    w4 = conv_w.reshape(8, 4, D, O)     # [v2, r, d, o], tap = 4*v2 + r
    W2 = np.stack([
        w4[:, 0] + w4[:, 1] + w4[:, 2] + w4[:, 3],   # (A,A2)
        w4[:, 0] + w4[:, 1],                         # (A,B2)
        w4[:, 2] + w4[:, 3],                         # (A,C2)
        w4[:, 0] + w4[:, 2],                         # (B,A2)
        w4[:, 0],                                    # (B,B2)
        w4[:, 2],                                    # (B,C2)
        w4[:, 1] + w4[:, 3],                         # (C,A2)
        w4[:, 1],                                    # (C,B2)
        w4[:, 3],                                    # (C,C2)
    ])                                               # [9, 8, D, O]
    # -> cw[m, p, c, k2=(v2,dh), o]
    cw2 = np.ascontiguousarray(
        W2.reshape(NCORR, 8, 2, 128, 4, 128).transpose(4, 3, 0, 1, 2, 5)
        .reshape(4, 128, NCORR, K2, 128)).astype(BF)
    b_c = np.ascontiguousarray(b).astype(np.float32)
    cb_c = np.ascontiguousarray(conv_b).astype(np.float32)

    in_maps = []
    for c in range(NCORES):
        in_maps.append({
            "xpt": xpt[c * BPC:(c + 1) * BPC],
            "w": wg, "b": b_c, "cw": cw2, "cb": cb_c,
        })
    res = run_bass_kernel_spmd(nc, in_maps, core_ids=list(range(NCORES)),
                               trace=TRACE)
    if TRACE and res.exec_time_ns is not None:
        print(f"HW exec time: {res.exec_time_ns} ns")
    y = np.concatenate([r["y"] for r in res.results], axis=0)
    return np.ascontiguousarray(y.transpose(0, 2, 1).astype(np.float32))


# revision 8
# speedup vs baseline: 1.7551x; 1.0220x over previous
"""Trainium2 Bass kernel for nn_LocallyDense: gather -> 16 group-GEMMs -> Conv1D(k=32) -> LeakyReLU.

Data-parallel over batch (32 -> 4 per core). Host applies the idx permutation;
stage 1 computes h = x_perm @ W_g per group in bf16.

The k=32 conv is computed with a 2-level Karatsuba decomposition of the tap
dimension: instead of 32 tap-GEMMs per output, 9 sub-correlations of 8 taps
over stride-4 subsampled/differenced sequences (datasets), cutting tensor-
engine work to 9/16 of direct. Derivation (per d-channel, position t):
  level 1: E_s=h[2s+1], P_s=h[2s]-h[2s+1], Q_s=h[2s+2]-h[2s+1]
           A=corr(E,a), B=corr(P,e), C=corr(Q,o) with a=w0+w1 pairs, e/o =
           even/odd taps; y_2u=A_u+B_u, y_2u+1=A_u+C_u
  level 2: the same split applied to each of A,B,C -> 9 corrs of 8 taps.
The 6 needed sums R[i][A2]+R[i][B2], R[i][A2]+R[i][C2] are built in PSUM by
snapshotting the A2 partial (ACT copy psA->psB) and accumulating B2/C2 on
top, so no extra matmul work. Final y phases are DVE adds of PSUM pairs with
the conv bias fused, then LeakyReLU, all in bf16 (rel err ~6e-3 << 2e-2).

DMA transfers are serialized in the timeline model: inputs stream on the SP
queue in execution-priority order; y outputs go out on the Activation queue
to avoid head-of-line blocking. Conv combo weights (2.25x the direct conv
weights) are streamed per (batch-pair, m, corr) and re-fetched for the second
batch pair to bound SBUF.
"""
import numpy as np
import ml_dtypes

import concourse.bass as bass
import concourse.mybir as mybir
import concourse.tile as tile
from concourse.alu_op_type import AluOpType
from concourse import bacc
from concourse.bass_utils import run_bass_kernel_spmd

B, N, F, G, S, D = 32, 1024, 512, 16, 64, 256
KC, O = 32, 512
T = N - KC + 1             # 993
NCORES = 8
BPC = B // NCORES          # 4
NEG_SLOPE = 0.2
F32 = mybir.dt.float32
BF16 = mybir.dt.bfloat16
BF = ml_dtypes.bfloat16

FKT = F // 128             # 4
NP = 1032                  # padded h length
U2 = 249                   # level-2 correlation outputs (4*249 >= 993+3)
NCORR = 9
K2 = 16                    # chunks per corr: 8 taps x 2 d-halves

TRACE = False
_cache = {}


def _sv(t, start, step, n):
    """Strided column view [128, n]: cols start, start+step, ..."""
    return t[:, start:start + step * n].rearrange(
        "p (r f) -> p r f", f=step)[:, :, 0]


def _build():
    nc = bacc.Bacc("TRN2", target_bir_lowering=False, debug=False,
                   num_devices=NCORES)
    xpt_d = nc.dram_tensor("xpt", [BPC, FKT, 128, N], BF16,
                           kind="ExternalInput").ap()
    w_d = nc.dram_tensor("w", [G, 128, FKT, D], BF16,
                         kind="ExternalInput").ap()
    b_d = nc.dram_tensor("b", [G, D], F32, kind="ExternalInput").ap()
    cw_d = nc.dram_tensor("cw", [4, 128, NCORR, K2, 128], BF16,
                          kind="ExternalInput").ap()
    cb_d = nc.dram_tensor("cb", [O], F32, kind="ExternalInput").ap()
    y_d = nc.dram_tensor("y", [BPC, O, T], BF16, kind="ExternalOutput").ap()

    with tile.TileContext(nc) as tc:
        with tc.tile_pool(name="xpt", bufs=2 * FKT) as p_xpt, \
             tc.tile_pool(name="wg", bufs=G) as p_w, \
             tc.tile_pool(name="ht", bufs=BPC * 2) as p_ht, \
             tc.tile_pool(name="pq", bufs=BPC * 4) as p_pq, \
             tc.tile_pool(name="df", bufs=BPC * 12) as p_df, \
             tc.tile_pool(name="bias", bufs=2) as p_bias, \
             tc.tile_pool(name="cw", bufs=12) as p_cw, \
             tc.tile_pool(name="ss", bufs=16) as p_ss, \
             tc.tile_pool(name="ys", bufs=3) as p_ys, \
             tc.tile_pool(name="yo", bufs=3) as p_yo:

            # ---------------- input DMAs (SP queue, priority order) --------
            b_sb = p_bias.tile([128, G * 2], F32)
            nc.sync.dma_start(b_sb[:], b_d.rearrange("g (m p) -> p (g m)", p=128))
            xpt_sb = {}

            def fetch_xpt(bb):
                for kt in range(FKT):
                    t_ = p_xpt.tile([128, N], BF16, tag="xpt",
                                    name=f"xpt{bb}_{kt}")
                    nc.sync.dma_start(t_[:], xpt_d[bb, kt])
                    xpt_sb[(bb, kt)] = t_

            fetch_xpt(0)
            w_sb = [p_w.tile([128, FKT, D], BF16, tag="wg", name=f"w{g}")
                    for g in range(G)]
            for g in range(G):
                nc.sync.dma_start(w_sb[g][:], w_d[g])

            # conv combo weights streamed in consumption order (A2 sets first)
            CORDER = [0, 3, 6, 1, 4, 7, 2, 5, 8]
            cw_sb = {}

            def fetch_cw(pair, m, crange):
                tiles = cw_sb.setdefault((pair, m), {})
                for c in crange:
                    t_ = p_cw.tile([128, K2, 128], BF16, tag="cw",
                                   name=f"cw{pair}_{m}_{c}")
                    nc.sync.dma_start(t_[:], cw_d[m][:, c])
                    tiles[c] = t_

            fetch_cw(0, 0, CORDER[:3])
            fetch_xpt(1)
            fetch_cw(0, 0, CORDER[3:6])
            cb_sb = p_bias.tile([128, 4], F32)
            nc.sync.dma_start(cb_sb[:], cb_d.rearrange("(m p) -> p m", p=128))
            fetch_cw(0, 0, CORDER[6:])
            fetch_xpt(2)
            fetch_xpt(3)
            for m in range(1, 4):
                fetch_cw(0, m, CORDER)
            for m in range(4):
                fetch_cw(1, m, CORDER)

            # ---------------- per-batch state ------------------------------
            ht = {}     # (bb, dh) -> [128, NP] bf16 (padded h)
            pq = {}     # (bb, dh, 0/1) -> P/Q [128, 514]
            dfs = {}    # (bb, dh, i, j) -> diff tiles [128, 256], j in 1,2

            def stage1(bb, p_ps1):
                for dh in range(2):
                    t_ = p_ht.tile([128, NP], BF16, tag="ht",
                                   name=f"ht{bb}_{dh}")
                    ht[(bb, dh)] = t_
                    nc.vector.memset(t_[:, N:NP], 0.0)
                for g in range(G):
                    for dh in range(2):
                        ps_full = p_ps1.tile([128, 256], F32, tag="ps2",
                                             name=f"ps1_{bb}_{g}_{dh}")
                        ps = ps_full[:, :S]
                        for kt in range(FKT):
                            nc.tensor.matmul(
                                ps, w_sb[g][:, kt, dh * 128:(dh + 1) * 128],
                                xpt_sb[(bb, kt)][:, g * S:(g + 1) * S],
                                start=(kt == 0), stop=(kt == FKT - 1))
                        nc.scalar.activation(
                            ht[(bb, dh)][:, g * S:(g + 1) * S], ps,
                            mybir.ActivationFunctionType.Identity,
                            bias=b_sb[:, g * 2 + dh: g * 2 + dh + 1])

            def sub(out, a, b_):
                nc.vector.scalar_tensor_tensor(
                    out, a, 1.0, b_, AluOpType.mult, AluOpType.subtract)

            def transforms(bb):
                for dh in range(2):
                    h_ = ht[(bb, dh)]
                    P = p_pq.tile([128, 514], BF16, tag="pq",
                                  name=f"P{bb}_{dh}")
                    Q = p_pq.tile([128, 514], BF16, tag="pq",
                                  name=f"Q{bb}_{dh}")
                    sub(P[:], _sv(h_, 0, 2, 514), _sv(h_, 1, 2, 514))
                    sub(Q[:], _sv(h_, 2, 2, 514), _sv(h_, 1, 2, 514))
                    pq[(bb, dh, 0)] = P
                    pq[(bb, dh, 1)] = Q
                    # level-2 diff datasets per lineage: (i, j=1 (B2), j=2 (C2))
                    for i, src, st0, stp in ((0, h_, 1, 4), (1, P, 0, 2),
                                             (2, Q, 0, 2)):
                        for j, off in ((1, 0), (2, 2 * (stp // 2))):
                            dt_ = p_df.tile([128, 256], BF16, tag="df",
                                            name=f"d{bb}_{dh}_{i}_{j}")
                            if i == 0:
                                va = _sv(src, 1 if j == 1 else 5, 4, 256)
                                vb = _sv(src, 3, 4, 256)
                            else:
                                va = _sv(src, 0 if j == 1 else 2, 2, 256)
                                vb = _sv(src, 1, 2, 256)
                            sub(dt_[:], va, vb)
                            dfs[(bb, dh, i, j)] = dt_

            def rhs(bb, dh, i, j, v2):
                if j == 0:
                    if i == 0:
                        return _sv(ht[(bb, dh)], 3 + 4 * v2, 4, U2)
                    return _sv(pq[(bb, dh, i - 1)], 1 + 2 * v2, 2, U2)
                return dfs[(bb, dh, i, j)][:, v2:v2 + U2]

            def s2_block(pair, m, bb, p_ps2):
                """9 GEMM-sets for one (batch, m). psA_i accumulates A2 then
                C2 (odd sums); A2 partials are snapshotted to SBUF (sA2_i)
                before C2 lands; B2 runs as its own clean PSUM group. Each
                PSUM bank is written only by the PE, so there are no
                cross-engine PSUM write races. Phases are assembled on DVE
                with at most one PSUM operand per op, conv bias fused."""
                cwt = cw_sb[(pair, m)]  # dict c -> tile
                psA, psB, sA2 = [], [], []
                for i in range(3):
                    ps = p_ps2.tile([128, 256], F32, tag="ps2",
                                    name=f"psA{pair}_{m}_{bb}_{i}")
                    psA.append(ps)
                    c = 3 * i + 0
                    for k2 in range(K2):
                        v2, dh = k2 // 2, k2 % 2
                        nc.tensor.matmul(
                            ps[:, :U2], cwt[c][:, k2, :], rhs(bb, dh, i, 0, v2),
                            start=(k2 == 0), stop=False, skip_group_check=True)
                    s_ = p_ss.tile([128, 256], F32, tag="ss",
                                   name=f"sA2_{pair}_{m}_{bb}_{i}")
                    nc.scalar.activation(s_[:, :U2], ps[:, :U2],
                                         mybir.ActivationFunctionType.Copy)
                    sA2.append(s_)
                for i in range(3):
                    pb = p_ps2.tile([128, 256], F32, tag="ps2",
                                    name=f"psB{pair}_{m}_{bb}_{i}")
                    psB.append(pb)
                    c = 3 * i + 1
                    for k2 in range(K2):
                        v2, dh = k2 // 2, k2 % 2
                        nc.tensor.matmul(
                            pb[:, :U2], cwt[c][:, k2, :], rhs(bb, dh, i, 1, v2),
                            start=(k2 == 0), stop=(k2 == K2 - 1))
                for i in range(3):
                    c = 3 * i + 2
                    for k2 in range(K2):
                        v2, dh = k2 // 2, k2 % 2
                        nc.tensor.matmul(
                            psA[i][:, :U2], cwt[c][:, k2, :],
                            rhs(bb, dh, i, 2, v2),
                            start=False, stop=(k2 == K2 - 1),
                            skip_group_check=True)
                # sAC = psA[0] (used twice; DVE allows only one PSUM operand)
                sAC = p_ss.tile([128, 256], F32, tag="ss",
                                name=f"sAC{pair}_{m}_{bb}")
                nc.scalar.activation(sAC[:, :U2], psA[0][:, :U2],
                                     mybir.ActivationFunctionType.Copy)
                # phases: y4u+0 = (sA2_0+sA2_1) + B2_0 + B2_1
                #         y4u+1 = (sA2_0+sA2_2) + B2_0 + B2_2
                #         y4u+2 = (A2_0+C2_0) + (A2_1+C2_1) = sAC + psA[1]
                #         y4u+3 = sAC + psA[2]
                y_sb = p_ys.tile([128, 1000], F32, tag="ys")
                cbm = cb_sb[:, m:m + 1]

                def stt(out, a, scal, b_):
                    nc.vector.scalar_tensor_tensor(
                        out, a, scal, b_, AluOpType.add, AluOpType.add)

                tAB = p_ss.tile([128, 256], F32, tag="ss",
                                name=f"tAB{pair}_{m}_{bb}")
                tAC = p_ss.tile([128, 256], F32, tag="ss",
                                name=f"tAC{pair}_{m}_{bb}")
                stt(tAB[:, :U2], sA2[0][:, :U2], 0.0, sA2[1][:, :U2])
                stt(tAC[:, :U2], sA2[0][:, :U2], 0.0, sA2[2][:, :U2])
                v0 = p_ss.tile([128, 256], F32, tag="ss",
                               name=f"v0_{pair}_{m}_{bb}")
                v1 = p_ss.tile([128, 256], F32, tag="ss",
                               name=f"v1_{pair}_{m}_{bb}")
                stt(v0[:, :U2], tAB[:, :U2], cbm, psB[0][:, :U2])
                stt(_sv(y_sb, 0, 4, U2), v0[:, :U2], 0.0, psB[1][:, :U2])
                stt(v1[:, :U2], tAC[:, :U2], cbm, psB[0][:, :U2])
                stt(_sv(y_sb, 1, 4, U2), v1[:, :U2], 0.0, psB[2][:, :U2])
                stt(_sv(y_sb, 2, 4, U2), sAC[:, :U2], cbm, psA[1][:, :U2])
                stt(_sv(y_sb, 3, 4, U2), sAC[:, :U2], cbm, psA[2][:, :U2])
                y_out = p_yo.tile([128, 1000], BF16, tag="yo")
                nc.vector.scalar_tensor_tensor(
                    y_out[:, :T], y_sb[:, :T], NEG_SLOPE, y_sb[:, :T],
                    AluOpType.mult, AluOpType.max)
                nc.scalar.dma_start(y_d[bb, m * 128:(m + 1) * 128, :],
                                    y_out[:, :T])

            with tc.tile_pool(name="ps2", bufs=8, space="PSUM") as p_ps2:
                stage1(0, p_ps2)
                transforms(0)
                s2_block(0, 0, 0, p_ps2)
                stage1(1, p_ps2)
                transforms(1)
                s2_block(0, 0, 1, p_ps2)
                stage1(2, p_ps2)
                transforms(2)
                s2_block(0, 1, 0, p_ps2)
                stage1(3, p_ps2)
                transforms(3)
                s2_block(0, 1, 1, p_ps2)
                for m in range(2, 4):
                    s2_block(0, m, 0, p_ps2)
                    s2_block(0, m, 1, p_ps2)
                for m in range(4):
                    s2_block(1, m, 2, p_ps2)
                    s2_block(1, m, 3, p_ps2)
    nc.compile()
    return nc


def kernel(x, idx, W, b, conv_w, conv_b):
    x = np.asarray(x); idx = np.asarray(idx); W = np.asarray(W)
    b = np.asarray(b); conv_w = np.asarray(conv_w); conv_b = np.asarray(conv_b)
    if "nc" not in _cache:
        _cache["nc"] = _build()
    nc = _cache["nc"]

    idx_flat = idx.reshape(-1).astype(np.int64)
    xpt = np.ascontiguousarray(
        x[:, idx_flat, :].transpose(0, 2, 1).reshape(B, FKT, 128, N)
    ).astype(BF)
    wg = np.ascontiguousarray(
        W.reshape(G, FKT, 128, D).transpose(0, 2, 1, 3)).astype(BF)

    # 9 combo-weight sets, 8 taps each (Karatsuba level-2 tap combos)
    w4 = conv_w.reshape(8, 4, D, O)     # [v2, r, d, o], tap = 4*v2 + r
    W2 = np.stack([
        w4[:, 0] + w4[:, 1] + w4[:, 2] + w4[:, 3],   # (A,A2)
        w4[:, 0] + w4[:, 1],                         # (A,B2)
        w4[:, 2] + w4[:, 3],                         # (A,C2)
        w4[:, 0] + w4[:, 2],                         # (B,A2)
        w4[:, 0],                                    # (B,B2)
        w4[:, 2],                                    # (B,C2)
        w4[:, 1] + w4[:, 3],                         # (C,A2)
        w4[:, 1],                                    # (C,B2)
        w4[:, 3],                                    # (C,C2)
    ])                                               # [9, 8, D, O]
    # -> cw[m, p, c, k2=(v2,dh), o]
    cw2 = np.ascontiguousarray(
        W2.reshape(NCORR, 8, 2, 128, 4, 128).transpose(4, 3, 0, 1, 2, 5)
        .reshape(4, 128, NCORR, K2, 128)).astype(BF)
    b_c = np.ascontiguousarray(b).astype(np.float32)
    cb_c = np.ascontiguousarray(conv_b).astype(np.float32)

    in_maps = []
    for c in range(NCORES):
        in_maps.append({
            "xpt": xpt[c * BPC:(c + 1) * BPC],
            "w": wg, "b": b_c, "cw": cw2, "cb": cb_c,
        })
    res = run_bass_kernel_spmd(nc, in_maps, core_ids=list(range(NCORES)),
                               trace=TRACE)
    if TRACE and res.exec_time_ns is not None:
        print(f"HW exec time: {res.exec_time_ns} ns")
    y = np.concatenate([r["y"] for r in res.results], axis=0)
    return np.ascontiguousarray(y.transpose(0, 2, 1).astype(np.float32))


# revision 9
# speedup vs baseline: 1.8134x; 1.0332x over previous
"""Trainium2 Bass kernel for nn_LocallyDense: gather -> 16 group-GEMMs -> Conv1D(k=32) -> LeakyReLU.

Data-parallel over batch (32 -> 4 per core). Host applies the idx permutation;
stage 1 computes h = x_perm @ W_g per group in bf16.

The k=32 conv is computed with a 2-level Karatsuba decomposition of the tap
dimension: instead of 32 tap-GEMMs per output, 9 sub-correlations of 8 taps
over stride-4 subsampled/differenced sequences (datasets), cutting tensor-
engine work to 9/16 of direct. Derivation (per d-channel, position t):
  level 1: E_s=h[2s+1], P_s=h[2s]-h[2s+1], Q_s=h[2s+2]-h[2s+1]
           A=corr(E,a), B=corr(P,e), C=corr(Q,o) with a=w0+w1 pairs, e/o =
           even/odd taps; y_2u=A_u+B_u, y_2u+1=A_u+C_u
  level 2: the same split applied to each of A,B,C -> 9 corrs of 8 taps.
The 6 needed sums R[i][A2]+R[i][B2], R[i][A2]+R[i][C2] are built in PSUM by
snapshotting the A2 partial (ACT copy psA->psB) and accumulating B2/C2 on
top, so no extra matmul work. Final y phases are DVE adds of PSUM pairs with
the conv bias fused, then LeakyReLU, all in bf16 (rel err ~6e-3 << 2e-2).

DMA transfers are serialized in the timeline model: inputs stream on the SP
queue in execution-priority order; y outputs go out on the Activation queue
to avoid head-of-line blocking. Conv combo weights (2.25x the direct conv
weights) are streamed per (batch-pair, m, corr) and re-fetched for the second
batch pair to bound SBUF.
"""
import numpy as np
import ml_dtypes

import concourse.bass as bass
import concourse.mybir as mybir
import concourse.tile as tile
from concourse.alu_op_type import AluOpType
from concourse import bacc
from concourse.bass_utils import run_bass_kernel_spmd

B, N, F, G, S, D = 32, 1024, 512, 16, 64, 256
KC, O = 32, 512
T = N - KC + 1             # 993
NCORES = 8
BPC = B // NCORES          # 4
NEG_SLOPE = 0.2
F32 = mybir.dt.float32
BF16 = mybir.dt.bfloat16
BF = ml_dtypes.bfloat16

FKT = F // 128             # 4
NP = 1032                  # padded h length
U2 = 249                   # level-2 correlation outputs (4*249 >= 993+3)
NCORR = 9
K2 = 16                    # chunks per corr: 8 taps x 2 d-halves

TRACE = False
_cache = {}


def _sv(t, start, step, n):
    """Strided column view [128, n]: cols start, start+step, ..."""
    return t[:, start:start + step * n].rearrange(
        "p (r f) -> p r f", f=step)[:, :, 0]


def _build():
    nc = bacc.Bacc("TRN2", target_bir_lowering=False, debug=False,
                   num_devices=NCORES)
    xpt_d = nc.dram_tensor("xpt", [BPC, FKT, 128, N], BF16,
                           kind="ExternalInput").ap()
    w_d = nc.dram_tensor("w", [G, 128, FKT, D], BF16,
                         kind="ExternalInput").ap()
    b_d = nc.dram_tensor("b", [G, D], F32, kind="ExternalInput").ap()
    cw_d = nc.dram_tensor("cw", [4, 128, NCORR, K2, 128], BF16,
                          kind="ExternalInput").ap()
    cb_d = nc.dram_tensor("cb", [O], F32, kind="ExternalInput").ap()
    y_d = nc.dram_tensor("y", [BPC, O, T], BF16, kind="ExternalOutput").ap()

    with tile.TileContext(nc) as tc:
        with tc.tile_pool(name="xpt", bufs=2 * FKT) as p_xpt, \
             tc.tile_pool(name="wg", bufs=G) as p_w, \
             tc.tile_pool(name="ht", bufs=BPC * 2) as p_ht, \
             tc.tile_pool(name="pq", bufs=BPC * 4) as p_pq, \
             tc.tile_pool(name="df", bufs=BPC * 12) as p_df, \
             tc.tile_pool(name="bias", bufs=2) as p_bias, \
             tc.tile_pool(name="cw", bufs=12) as p_cw, \
             tc.tile_pool(name="ss", bufs=16) as p_ss, \
             tc.tile_pool(name="ys", bufs=3) as p_ys, \
             tc.tile_pool(name="yo", bufs=3) as p_yo:

            # ---------------- input DMAs (SP queue, priority order) --------
            b_sb = p_bias.tile([128, G * 2], F32)
            nc.sync.dma_start(b_sb[:], b_d.rearrange("g (m p) -> p (g m)", p=128))
            xpt_sb = {}

            def fetch_xpt(bb):
                for kt in range(FKT):
                    t_ = p_xpt.tile([128, N], BF16, tag="xpt",
                                    name=f"xpt{bb}_{kt}")
                    nc.sync.dma_start(t_[:], xpt_d[bb, kt])
                    xpt_sb[(bb, kt)] = t_

            fetch_xpt(0)
            w_sb = [p_w.tile([128, FKT, D], BF16, tag="wg", name=f"w{g}")
                    for g in range(G)]
            for g in range(G):
                nc.sync.dma_start(w_sb[g][:], w_d[g])

            # conv combo weights streamed in consumption order (A2 sets first)
            CORDER = [0, 3, 6, 1, 4, 7, 2, 5, 8]
            cw_sb = {}

            def fetch_cw(pair, m, crange):
                tiles = cw_sb.setdefault((pair, m), {})
                for c in crange:
                    t_ = p_cw.tile([128, K2, 128], BF16, tag="cw",
                                   name=f"cw{pair}_{m}_{c}")
                    nc.sync.dma_start(t_[:], cw_d[m][:, c])
                    tiles[c] = t_

            fetch_cw(0, 0, CORDER[:3])
            fetch_xpt(1)
            fetch_cw(0, 0, CORDER[3:6])
            cb_sb = p_bias.tile([128, 4], F32)
            nc.sync.dma_start(cb_sb[:], cb_d.rearrange("(m p) -> p m", p=128))
            fetch_cw(0, 0, CORDER[6:])
            fetch_xpt(2)
            fetch_xpt(3)
            for m in range(1, 4):
                fetch_cw(0, m, CORDER)
            for m in range(4):
                fetch_cw(1, m, CORDER)

            # ---------------- per-batch state ------------------------------
            ht = {}     # (bb, dh) -> [128, NP] bf16 (padded h)
            pq = {}     # (bb, dh, 0/1) -> P/Q [128, 514]
            dfs = {}    # (bb, dh, i, j) -> diff tiles [128, 256], j in 1,2

            def stage1(bb, p_ps1):
                for dh in range(2):
                    t_ = p_ht.tile([128, NP], BF16, tag="ht",
                                   name=f"ht{bb}_{dh}")
                    ht[(bb, dh)] = t_
                    nc.vector.memset(t_[:, N:NP], 0.0)
                for g in range(G):
                    for dh in range(2):
                        ps_full = p_ps1.tile([128, 256], F32, tag="ps2",
                                             name=f"ps1_{bb}_{g}_{dh}")
                        ps = ps_full[:, :S]
                        for kt in range(FKT):
                            nc.tensor.matmul(
                                ps, w_sb[g][:, kt, dh * 128:(dh + 1) * 128],
                                xpt_sb[(bb, kt)][:, g * S:(g + 1) * S],
                                start=(kt == 0), stop=(kt == FKT - 1))
                        if dh == 0:
                            nc.scalar.activation(
                                ht[(bb, dh)][:, g * S:(g + 1) * S], ps,
                                mybir.ActivationFunctionType.Identity,
                                bias=b_sb[:, g * 2 + dh: g * 2 + dh + 1])
                        else:
                            nc.vector.tensor_scalar_add(
                                ht[(bb, dh)][:, g * S:(g + 1) * S], ps,
                                b_sb[:, g * 2 + dh: g * 2 + dh + 1])

            def sub(out, a, b_):
                nc.vector.scalar_tensor_tensor(
                    out, a, 1.0, b_, AluOpType.mult, AluOpType.subtract)

            def transforms(bb):
                for dh in range(2):
                    h_ = ht[(bb, dh)]
                    P = p_pq.tile([128, 514], BF16, tag="pq",
                                  name=f"P{bb}_{dh}")
                    Q = p_pq.tile([128, 514], BF16, tag="pq",
                                  name=f"Q{bb}_{dh}")
                    sub(P[:], _sv(h_, 0, 2, 514), _sv(h_, 1, 2, 514))
                    sub(Q[:], _sv(h_, 2, 2, 514), _sv(h_, 1, 2, 514))
                    pq[(bb, dh, 0)] = P
                    pq[(bb, dh, 1)] = Q
                    # level-2 diff datasets per lineage: (i, j=1 (B2), j=2 (C2))
                    for i, src, st0, stp in ((0, h_, 1, 4), (1, P, 0, 2),
                                             (2, Q, 0, 2)):
                        for j, off in ((1, 0), (2, 2 * (stp // 2))):
                            dt_ = p_df.tile([128, 256], BF16, tag="df",
                                            name=f"d{bb}_{dh}_{i}_{j}")
                            if i == 0:
                                va = _sv(src, 1 if j == 1 else 5, 4, 256)
                                vb = _sv(src, 3, 4, 256)
                            else:
                                va = _sv(src, 0 if j == 1 else 2, 2, 256)
                                vb = _sv(src, 1, 2, 256)
                            sub(dt_[:], va, vb)
                            dfs[(bb, dh, i, j)] = dt_

            def rhs(bb, dh, i, j, v2):
                if j == 0:
                    if i == 0:
                        return _sv(ht[(bb, dh)], 3 + 4 * v2, 4, U2)
                    return _sv(pq[(bb, dh, i - 1)], 1 + 2 * v2, 2, U2)
                return dfs[(bb, dh, i, j)][:, v2:v2 + U2]

            def s2_block(pair, m, bb, p_ps2, splits=((0, U2),)):
                """9 GEMM-sets for one (batch, m-tile), optionally split into
                column ranges so the drain chain of the final block overlaps
                its own GEMMs. psA_i accumulates A2 then C2; A2 partials are
                snapshotted to SBUF before C2 lands; B2 is its own clean PSUM
                group. Each PSUM bank has a single PE writer; phases are DVE
                ops with at most one PSUM operand, conv bias fused."""
                cwt = cw_sb[(pair, m)]  # dict c -> tile
                y_sb = p_ys.tile([128, 1000], F32, tag="ys")
                y_out = p_yo.tile([128, 1000], BF16, tag="yo")
                cbm = cb_sb[:, m:m + 1]

                def stt(out, a, scal, b_):
                    nc.vector.scalar_tensor_tensor(
                        out, a, scal, b_, AluOpType.add, AluOpType.add)

                for si, (u0, u1) in enumerate(splits):
                    L = u1 - u0
                    sfx = f"{pair}_{m}_{bb}_{si}"
                    psA, psB, sA2 = [], [], []
                    for i in range(3):
                        ps = p_ps2.tile([128, 256], F32, tag="ps2",
                                        name=f"psA{sfx}_{i}")
                        psA.append(ps)
                        c = 3 * i + 0
                        for k2 in range(K2):
                            v2, dh = k2 // 2, k2 % 2
                            nc.tensor.matmul(
                                ps[:, :L], cwt[c][:, k2, :],
                                rhs(bb, dh, i, 0, v2)[:, u0:u1],
                                start=(k2 == 0), stop=False,
                                skip_group_check=True)
                        s_ = p_ss.tile([128, 256], F32, tag="ss",
                                       name=f"sA2_{sfx}_{i}")
                        nc.scalar.activation(s_[:, :L], ps[:, :L],
                                             mybir.ActivationFunctionType.Copy)
                        sA2.append(s_)
                    for i in range(3):
                        pb = p_ps2.tile([128, 256], F32, tag="ps2",
                                        name=f"psB{sfx}_{i}")
                        psB.append(pb)
                        c = 3 * i + 1
                        for k2 in range(K2):
                            v2, dh = k2 // 2, k2 % 2
                            nc.tensor.matmul(
                                pb[:, :L], cwt[c][:, k2, :],
                                rhs(bb, dh, i, 1, v2)[:, u0:u1],
                                start=(k2 == 0), stop=(k2 == K2 - 1))
                    for i in range(3):
                        c = 3 * i + 2
                        for k2 in range(K2):
                            v2, dh = k2 // 2, k2 % 2
                            nc.tensor.matmul(
                                psA[i][:, :L], cwt[c][:, k2, :],
                                rhs(bb, dh, i, 2, v2)[:, u0:u1],
                                start=False, stop=(k2 == K2 - 1),
                                skip_group_check=True)
                    sAC = p_ss.tile([128, 256], F32, tag="ss",
                                    name=f"sAC{sfx}")
                    nc.scalar.activation(sAC[:, :L], psA[0][:, :L],
                                         mybir.ActivationFunctionType.Copy)
                    tAB = p_ss.tile([128, 256], F32, tag="ss",
                                    name=f"tAB{sfx}")
                    tAC = p_ss.tile([128, 256], F32, tag="ss",
                                    name=f"tAC{sfx}")
                    stt(tAB[:, :L], sA2[0][:, :L], 0.0, sA2[1][:, :L])
                    stt(tAC[:, :L], sA2[0][:, :L], 0.0, sA2[2][:, :L])
                    v0 = p_ss.tile([128, 256], F32, tag="ss", name=f"v0_{sfx}")
                    v1 = p_ss.tile([128, 256], F32, tag="ss", name=f"v1_{sfx}")
                    stt(v0[:, :L], tAB[:, :L], cbm, psB[0][:, :L])
                    stt(_sv(y_sb, 4 * u0 + 0, 4, L), v0[:, :L], 0.0,
                        psB[1][:, :L])
                    stt(v1[:, :L], tAC[:, :L], cbm, psB[0][:, :L])
                    stt(_sv(y_sb, 4 * u0 + 1, 4, L), v1[:, :L], 0.0,
                        psB[2][:, :L])
                    stt(_sv(y_sb, 4 * u0 + 2, 4, L), sAC[:, :L], cbm,
                        psA[1][:, :L])
                    stt(_sv(y_sb, 4 * u0 + 3, 4, L), sAC[:, :L], cbm,
                        psA[2][:, :L])
                    c0, c1 = 4 * u0, min(4 * u1, T)
                    nc.vector.scalar_tensor_tensor(
                        y_out[:, c0:c1], y_sb[:, c0:c1], NEG_SLOPE,
                        y_sb[:, c0:c1], AluOpType.mult, AluOpType.max)
                    nc.scalar.dma_start(
                        y_d[bb, m * 128:(m + 1) * 128, c0:c1],
                        y_out[:, c0:c1])

            with tc.tile_pool(name="ps2", bufs=8, space="PSUM") as p_ps2:
                stage1(0, p_ps2)
                transforms(0)
                s2_block(0, 0, 0, p_ps2)
                stage1(1, p_ps2)
                transforms(1)
                s2_block(0, 0, 1, p_ps2)
                stage1(2, p_ps2)
                transforms(2)
                s2_block(0, 1, 0, p_ps2)
                stage1(3, p_ps2)
                transforms(3)
                s2_block(0, 1, 1, p_ps2)
                for m in range(2, 4):
                    s2_block(0, m, 0, p_ps2)
                    s2_block(0, m, 1, p_ps2)
                for m in range(4):
                    s2_block(1, m, 2, p_ps2)
                    if m < 3:
                        s2_block(1, m, 3, p_ps2)
                s2_block(1, 3, 3, p_ps2, splits=((0, 160), (160, U2)))
    nc.compile()
    return nc


def kernel(x, idx, W, b, conv_w, conv_b):
    x = np.asarray(x); idx = np.asarray(idx); W = np.asarray(W)
    b = np.asarray(b); conv_w = np.asarray(conv_w); conv_b = np.asarray(conv_b)
    if "nc" not in _cache:
        _cache["nc"] = _build()
    nc = _cache["nc"]

    idx_flat = idx.reshape(-1).astype(np.int64)
    xpt = np.ascontiguousarray(
        x[:, idx_flat, :].transpose(0, 2, 1).reshape(B, FKT, 128, N)
    ).astype(BF)
    wg = np.ascontiguousarray(
        W.reshape(G, FKT, 128, D).transpose(0, 2, 1, 3)).astype(BF)

    # 9 combo-weight sets, 8 taps each (Karatsuba level-2 tap combos)
    w4 = conv_w.reshape(8, 4, D, O)     # [v2, r, d, o], tap = 4*v2 + r
    W2 = np.stack([
        w4[:, 0] + w4[:, 1] + w4[:, 2] + w4[:, 3],   # (A,A2)
        w4[:, 0] + w4[:, 1],                         # (A,B2)
        w4[:, 2] + w4[:, 3],                         # (A,C2)
        w4[:, 0] + w4[:, 2],                         # (B,A2)
        w4[:, 0],                                    # (B,B2)
        w4[:, 2],                                    # (B,C2)
        w4[:, 1] + w4[:, 3],                         # (C,A2)
        w4[:, 1],                                    # (C,B2)
        w4[:, 3],                                    # (C,C2)
    ])                                               # [9, 8, D, O]
    # -> cw[m, p, c, k2=(v2,dh), o]
    cw2 = np.ascontiguousarray(
        W2.reshape(NCORR, 8, 2, 128, 4, 128).transpose(4, 3, 0, 1, 2, 5)
        .reshape(4, 128, NCORR, K2, 128)).astype(BF)
    b_c = np.ascontiguousarray(b).astype(np.float32)
    cb_c = np.ascontiguousarray(conv_b).astype(np.float32)

    in_maps = []
    for c in range(NCORES):
        in_maps.append({
            "xpt": xpt[c * BPC:(c + 1) * BPC],
            "w": wg, "b": b_c, "cw": cw2, "cb": cb_c,
        })
    res = run_bass_kernel_spmd(nc, in_maps, core_ids=list(range(NCORES)),
                               trace=TRACE)
    if TRACE and res.exec_time_ns is not None:
        print(f"HW exec time: {res.exec_time_ns} ns")
    y = np.concatenate([r["y"] for r in res.results], axis=0)
    return np.ascontiguousarray(y.transpose(0, 2, 1).astype(np.float32))
